# revision 31
# baseline (speedup 1.0000x reference)
"""Self-contained Trainium2 Bass kernel: pre-LN multi-head attention block.

Computes, for x [B=8, S=1024, D=1024] (fp32) and packed attention weights:
    out = x + out_proj(MHA(LayerNorm(x)))
matching torch nn.MultiheadAttention's explicit (non-flash) path with 16 heads.

Sharding: data-parallel over batch — core i handles batch element i; no
collectives, outputs are concatenated on the host.

Per-core layout strategy (transposed activations, d on partitions):
  - host supplies xT = x[i].T (bf16) so LN + projections run with the
    contraction dim (d) on SBUF partitions; LN stats (sums over d) are
    ones-vector matmuls on the PE; the per-token mean/rstd rows are
    broadcast across partitions via a DRAM round-trip DMA.
  - QKV projection emits Q^T,K^T (head dim on partitions) and V in natural
    layout [t, e'], each head's V augmented with a ones column so the
    PV matmul also produces the softmax denominator (PSUM row 64).
  - scores^T[t, s] = K^T.T @ Q^T per head (K=64 contraction, two heads
    row-packed via tile_position); softmax exp runs on the scalar engine
    straight out of PSUM with the 1/sqrt(dh) scale folded in; no max
    subtraction (scores are O(few) by construction).
  - ctx^T normalization by 1/denominator is deferred to just before the
    out-projection (broadcast via DRAM round-trip); the out-projection
    lands in natural [s, e] layout for the residual add.
"""

import os

import numpy as np
import ml_dtypes

P = 128
D = 1024
H = 16
DH = 64
E = 3 * D
B = 8
S = 1024
LN_EPS = 1e-5
N_CORES = 8

_ND = D // P   # d tiles (8)
_NC = 512      # matmul moving chunk

LAST_RESULTS = None
_NC_CACHE = {}


def _emit(tc, aps, S_=S):
    import concourse.bass as bass
    from concourse import mybir

    nc = tc.nc
    f32 = mybir.dt.float32
    bf16 = mybir.dt.bfloat16
    FT = mybir.ActivationFunctionType
    OP = mybir.AluOpType

    ns = S_ // P
    ncs = max(1, S_ // _NC)
    NCK = min(_NC, S_)
    nqk = 2 * D // P  # q+k e-tiles (16)

    xT, xnat, winT, woutT, gammat, betat, binqk, binv, bout, out = (
        aps["xt"], aps["xnat"], aps["wint"], aps["woutt"], aps["gammat"],
        aps["betat"], aps["binqk"], aps["binv"], aps["bout"], aps["out"],
    )
    winT_r = winT.rearrange("(a p) e -> p a e", p=P)

    with tc.tile_pool(name="consts", bufs=1) as consts, \
         tc.tile_pool(name="acts", bufs=1) as acts, \
         tc.tile_pool(name="winv", bufs=1) as wvpool, \
         tc.tile_pool(name="dscratch", bufs=1, space="DRAM") as dscratch:

        # V-column weights (DMA issued after the x chunks, below)
        winv_sb = wvpool.tile([P, _ND, D], bf16, tag="w")

        # ---------- constants ----------
        cvec = consts.tile([P, _ND + _ND + nqk], f32)
        nc.sync.dma_start(out=cvec[:, 0:_ND], in_=gammat)
        nc.sync.dma_start(out=cvec[:, _ND:2 * _ND], in_=betat)
        nc.sync.dma_start(out=cvec[:, 2 * _ND:2 * _ND + nqk], in_=binqk)
        gamma_sb = cvec[:, 0:_ND]
        beta_sb = cvec[:, _ND:2 * _ND]
        binqk_sb = cvec[:, 2 * _ND:2 * _ND + nqk]
        ones_col = consts.tile([P, 1], bf16)
        nc.vector.memset(ones_col, 1.0)
        ones_row = consts.tile([1, P], bf16)
        nc.vector.memset(ones_row, 1.0)
        neg_ones_row = consts.tile([1, P], bf16)
        nc.vector.memset(neg_ones_row, -1.0)
        binv_bc = consts.tile([P, D], f32)
        nc.gpsimd.dma_start(out=binv_bc, in_=binv[None, :].to_broadcast((P, D)))
        bout_bc = consts.tile([P, D], f32)
        nc.gpsimd.dma_start(out=bout_bc, in_=bout[None, :].to_broadcast((P, D)))

        # ---------- persistent activations ----------
        xnT_sb = acts.tile([P, _ND, S_], bf16)      # normalized x, transposed
        qkT_sb = acts.tile([P, nqk, S_], bf16)      # q (tiles 0..7), k (8..15)
        v_sb = acts.tile([P, ns, H, DH + 1], bf16)  # v natural + ones column
        ctx_sb = acts.tile([P, _ND, S_], bf16)      # ctx^T, normalized in place
        # softmax 1/denominator: head h at partition 32*(h//4), slot h%4
        # (SBUF engine APs may only start at partitions 0/32/64/96)
        den_sb = acts.tile([P, 4, S_], bf16)
        rd_dram = dscratch.tile([H, S_], bf16)

        # ================= Phase 1: LayerNorm =================
        with tc.tile_pool(name="lnsb", bufs=1) as lnsb, \
             tc.tile_pool(name="lnrow", bufs=1) as lnrow, \
             tc.tile_pool(name="lntmp", bufs=2) as lntmp, \
             tc.tile_pool(name="lnps", bufs=1, space="PSUM") as lnps:
            xT_sb = lnsb.tile([P, _ND, S_], bf16)
            sx_ps = lnps.tile([1, S_], f32, tag="sx")
            sx2_ps = lnps.tile([1, S_], f32, tag="sx2")
            for j in range(_ND):
                nc.sync.dma_start(out=xT_sb[:, j],
                                  in_=xT.rearrange("(a p) s -> p a s", p=P)[:, j])
                sq = lntmp.tile([P, S_], bf16, tag="sq")
                with nc.allow_low_precision(reason="x^2 for LN stats in bf16"):
                    nc.vector.tensor_tensor(out=sq, in0=xT_sb[:, j],
                                            in1=xT_sb[:, j], op=OP.mult)
                for c in range(ncs):
                    sl = slice(c * NCK, (c + 1) * NCK)
                    nc.tensor.matmul(sx_ps[:, sl], lhsT=ones_col, rhs=xT_sb[:, j, sl],
                                     start=(j == 0), stop=(j == _ND - 1))
                    nc.tensor.matmul(sx2_ps[:, sl], lhsT=ones_col, rhs=sq[:, sl],
                                     start=(j == 0), stop=(j == _ND - 1))
            eps_sb = lnrow.tile([1, 1], f32)
            nc.vector.memset(eps_sb, LN_EPS)
            mu = lnrow.tile([1, S_], f32)
            nc.vector.tensor_scalar_mul(mu, sx_ps, 1.0 / D)
            t1 = lnrow.tile([1, S_], f32)
            nc.vector.tensor_tensor(out=t1, in0=mu, in1=mu, op=OP.mult)      # mu^2
            t2 = lnrow.tile([1, S_], f32)
            nc.vector.tensor_scalar_mul(t2, sx2_ps, 1.0 / D)
            nc.vector.tensor_tensor(out=t2, in0=t2, in1=t1, op=OP.subtract)  # var
            nc.scalar.activation(out=t1, in_=t2, func=FT.Sqrt, bias=eps_sb)  # std
            ab_bf = lnrow.tile([1, 2, S_], bf16)
            with nc.allow_low_precision(reason="LN shift/scale rows in bf16"):
                nc.vector.reciprocal(out=ab_bf[:, 1], in_=t1)                # B
                nc.vector.tensor_tensor(out=ab_bf[:, 0], in0=mu,
                                        in1=ab_bf[:, 1], op=OP.mult)         # mu*B
            # broadcast across partitions: K=1 matmuls (A gets -1 via lhsT)
            a_ps = lnps.tile([P, S_], f32, tag="ab", bufs=2)
            b_ps = lnps.tile([P, S_], f32, tag="ab", bufs=2)
            for c in range(ncs):
                sl = slice(c * NCK, (c + 1) * NCK)
                nc.tensor.matmul(a_ps[:, sl], lhsT=neg_ones_row, rhs=ab_bf[:, 0, sl],
                                 start=True, stop=True)
                nc.tensor.matmul(b_ps[:, sl], lhsT=ones_row, rhs=ab_bf[:, 1, sl],
                                 start=True, stop=True)
            nc.sync.dma_start(out=winv_sb, in_=winT_r[:, :, 2 * D:])
            # copy to SBUF promptly so the LN PSUM pool can release (the
            # attention pool reuses its banks; holding them would serialize)
            a_bc = lnsb.tile([P, S_], f32)
            nc.vector.tensor_copy(out=a_bc, in_=a_ps)
            b_bc = lnsb.tile([P, S_], f32)
            nc.vector.tensor_copy(out=b_bc, in_=b_ps)

            for j in range(_ND):
                t = lntmp.tile([P, S_], f32, tag="nrm")
                nc.vector.tensor_tensor(out=t, in0=xT_sb[:, j], in1=b_bc, op=OP.mult)
                nc.vector.tensor_tensor(out=t, in0=t, in1=a_bc, op=OP.add)
                nc.scalar.activation(out=xnT_sb[:, j], in_=t, func=FT.Identity,
                                     bias=beta_sb[:, j:j + 1], scale=gamma_sb[:, j:j + 1])

        # ============ Phases 2-4: projections + attention ============
        with tc.tile_pool(name="winqk", bufs=1) as wqpool, \
             tc.tile_pool(name="expp", bufs=3) as expp, \
             tc.tile_pool(name="mps", bufs=1, space="PSUM") as mps:

            winqk_sb = wqpool.tile([P, _ND, 2 * D], bf16)
            nc.sync.dma_start(out=winqk_sb, in_=winT_r[:, :, 0:2 * D])

            # ones column of the augmented V blocks
            nc.vector.memset(v_sb[:, :, :, DH:DH + 1], 1.0)

            # ---- V projection (natural layout [t, e']) ----
            for st in range(ns):
                ps = mps.tile([P, D], f32, tag="sc", bufs=2)
                for c in range(D // _NC):
                    sl = slice(c * _NC, (c + 1) * _NC)
                    for j in range(_ND):
                        nc.tensor.matmul(ps[:, sl], lhsT=xnT_sb[:, j, st * P:(st + 1) * P],
                                         rhs=winv_sb[:, j, sl],
                                         start=(j == 0), stop=(j == _ND - 1))
                nc.vector.tensor_tensor(
                    out=v_sb[:, st, :, 0:DH],
                    in0=ps.rearrange("p (h d) -> p h d", d=DH),
                    in1=binv_bc.rearrange("p (h d) -> p h d", d=DH),
                    op=OP.add)

            # out-proj weights reuse the winv slot (V projection is done with it)
            woutT_sb = wvpool.tile([P, _ND, D], bf16, tag="w")
            nc.sync.dma_start(out=woutT_sb, in_=woutT.rearrange("(a p) e -> p a e", p=P))

            # ---- per head-pair: Q/K projection, scores^T, exp, PV ----
            for hp in range(H // 2):
                for et in (hp, 8 + hp):
                    ps = mps.tile([P, S_], f32, tag="qs", bufs=1)
                    for c in range(ncs):
                        sl = slice(c * NCK, (c + 1) * NCK)
                        for j in range(_ND):
                            nc.tensor.matmul(ps[:, sl],
                                             lhsT=winqk_sb[:, j, et * P:(et + 1) * P],
                                             rhs=xnT_sb[:, j, sl],
                                             start=(j == 0), stop=(j == _ND - 1))
                    nc.vector.tensor_scalar_add(qkT_sb[:, et], ps, binqk_sb[:, et:et + 1])

                ex = [expp.tile([P, ns, S_], bf16, tag="exp", name=f"ex{hp}_{i}")
                      for i in range(2)]
                for tt in range(ns):
                    pss = [mps.tile([P, S_], f32, tag="sc", bufs=2,
                                    name=f"sc{hp}_{tt}_{i}") for i in range(2)]
                    for idx in range(2):
                        base = idx * DH
                        for c in range(ncs):
                            sl = slice(c * NCK, (c + 1) * NCK)
                            nc.tensor.matmul(
                                pss[idx][:, sl],
                                lhsT=qkT_sb[base:base + DH, 8 + hp, tt * P:(tt + 1) * P],
                                rhs=qkT_sb[base:base + DH, hp, sl],
                                start=True, stop=True, tile_position=(base, 0))
                    for idx in range(2):
                        nc.scalar.activation(out=ex[idx][:, tt], in_=pss[idx],
                                             func=FT.Exp, scale=0.125)

                # PV with ones-augmented V: rows 0..63 ctx^T, row 64 denominator
                for idx in range(2):
                    h = 2 * hp + idx
                    dp = 32 * (h // 4)
                    for c in range(ncs):
                        sl = slice(c * NCK, (c + 1) * NCK)
                        pv = mps.tile([DH + 1, NCK], f32, tag="pv", bufs=2)
                        for tt in range(ns):
                            nc.tensor.matmul(pv, lhsT=v_sb[:, tt, h, :],
                                             rhs=ex[idx][:, tt, sl],
                                             start=(tt == 0), stop=(tt == ns - 1))
                        nc.vector.tensor_copy(out=ctx_sb[idx * DH:(idx + 1) * DH, hp, sl],
                                              in_=pv[0:DH, :])
                        nc.vector.tensor_copy(out=den_sb[dp:dp + 1, h % 4, sl],
                                              in_=pv[DH:DH + 1, :])
                    # reciprocal + ship to DRAM for the later broadcast
                    with nc.allow_low_precision(reason="softmax denom in bf16"):
                        nc.vector.reciprocal(out=den_sb[dp:dp + 1, h % 4, :],
                                             in_=den_sb[dp:dp + 1, h % 4, :])
                    nc.sync.dma_start(out=rd_dram[h:h + 1, :],
                                      in_=den_sb[dp:dp + 1, h % 4, :])

        # ============ Phase 5: normalize + out-projection ============
        with tc.tile_pool(name="p5", bufs=2) as p5, \
             tc.tile_pool(name="p5ps", bufs=1, space="PSUM") as p5ps:
            for hp in range(H // 2):
                rdbc = p5.tile([P, S_], bf16, tag="rdbc")
                nc.sync.dma_start(out=rdbc[0:DH, :],
                                  in_=rd_dram[2 * hp][None, :].to_broadcast((DH, S_)))
                nc.sync.dma_start(out=rdbc[DH:P, :],
                                  in_=rd_dram[2 * hp + 1][None, :].to_broadcast((DH, S_)))
                nc.vector.tensor_tensor(out=ctx_sb[:, hp], in0=ctx_sb[:, hp],
                                        in1=rdbc, op=OP.mult)

            for st in range(ns):
                xn_t = p5.tile([P, D], f32, tag="xnat")
                nc.sync.dma_start(out=xn_t, in_=xnat[st * P:(st + 1) * P, :])
                nc.vector.tensor_tensor(out=xn_t, in0=xn_t, in1=bout_bc, op=OP.add)
                ot = p5.tile([P, D], f32, tag="out")
                for c in range(D // _NC):
                    sl = slice(c * _NC, (c + 1) * _NC)
                    po = p5ps.tile([P, _NC], f32, tag="po", bufs=4)
                    for j in range(_ND):
                        nc.tensor.matmul(po, lhsT=ctx_sb[:, j, st * P:(st + 1) * P],
                                         rhs=woutT_sb[:, j, sl],
                                         start=(j == 0), stop=(j == _ND - 1))
                    nc.vector.tensor_tensor(out=ot[:, sl], in0=po, in1=xn_t[:, sl], op=OP.add)
                    nc.sync.dma_start(out=out[st * P:(st + 1) * P, sl], in_=ot[:, sl])


def build_nc(S_=S):
    import concourse.bacc as bacc
    import concourse.tile as tile
    from concourse import mybir

    f32 = mybir.dt.float32
    bf16 = mybir.dt.bfloat16

    nc = bacc.Bacc("TRN2", target_bir_lowering=False, debug=False)
    aps = {
        "xt": nc.dram_tensor("xt", [D, S_], bf16, kind="ExternalInput").ap(),
        "xnat": nc.dram_tensor("xnat", [S_, D], f32, kind="ExternalInput").ap(),
        "wint": nc.dram_tensor("wint", [D, E], bf16, kind="ExternalInput").ap(),
        "woutt": nc.dram_tensor("woutt", [D, D], bf16, kind="ExternalInput").ap(),
        "gammat": nc.dram_tensor("gammat", [P, _ND], f32, kind="ExternalInput").ap(),
        "betat": nc.dram_tensor("betat", [P, _ND], f32, kind="ExternalInput").ap(),
        "binqk": nc.dram_tensor("binqk", [P, 2 * D // P], f32, kind="ExternalInput").ap(),
        "binv": nc.dram_tensor("binv", [D], f32, kind="ExternalInput").ap(),
        "bout": nc.dram_tensor("bout", [D], f32, kind="ExternalInput").ap(),
        "out": nc.dram_tensor("out", [S_, D], f32, kind="ExternalOutput").ap(),
    }
    with tile.TileContext(nc) as tc:
        _emit(tc, aps, S_)
    nc.compile()
    return nc


def prep_inputs(x, ln_gamma, ln_beta, in_proj_w, in_proj_b, out_proj_w, out_proj_b,
                S_=S, n_cores=N_CORES):
    bf = ml_dtypes.bfloat16
    f32c = lambda a: np.ascontiguousarray(a, dtype=np.float32)
    shared = {
        "wint": np.ascontiguousarray(np.asarray(in_proj_w, np.float32).T).astype(bf),
        "woutt": np.ascontiguousarray(np.asarray(out_proj_w, np.float32).T).astype(bf),
        "gammat": f32c(np.asarray(ln_gamma, np.float32).reshape(_ND, P).T),
        "betat": f32c(np.asarray(ln_beta, np.float32).reshape(_ND, P).T),
        "binqk": f32c(np.asarray(in_proj_b, np.float32)[:2 * D].reshape(2 * D // P, P).T),
        "binv": f32c(np.asarray(in_proj_b, np.float32)[2 * D:]),
        "bout": f32c(np.asarray(out_proj_b, np.float32)),
    }
    in_maps = []
    for i in range(n_cores):
        xi = np.asarray(x[i], np.float32)[:S_]
        m = dict(shared)
        m["xt"] = np.ascontiguousarray(xi.T).astype(bf)
        m["xnat"] = f32c(xi)
        in_maps.append(m)
    return in_maps


def kernel(x, ln_gamma, ln_beta, in_proj_w, in_proj_b, out_proj_w, out_proj_b):
    global LAST_RESULTS
    from concourse import bass_utils

    if "nc" not in _NC_CACHE:
        _NC_CACHE["nc"] = build_nc(S)
    nc = _NC_CACHE["nc"]

    in_maps = prep_inputs(x, ln_gamma, ln_beta, in_proj_w, in_proj_b,
                          out_proj_w, out_proj_b)
    res = bass_utils.run_bass_kernel_spmd(nc, in_maps, core_ids=list(range(N_CORES)))
    LAST_RESULTS = res
    out = np.stack([r["out"] for r in res.results], axis=0)
    return np.ascontiguousarray(out, dtype=np.float32)
